# revision 1
# baseline (speedup 1.0000x reference)
"""Trainium2 Bass kernel for KANCell: relu(sum(relu(x))) over 2**25 fp32.

Data-parallel sharded reduction: the flat vector is split across 8
NeuronCores. Each core streams its 16 MiB shard HBM->SBUF in 2 MiB tiles
and fuses relu + per-partition partial sum in a single ScalarE activation
(accum_out) per tile, overlapped with the next tile's DMA. Per-core
partials come back to the host, which does the final (tiny) sum + ReLU.

Raw bass (no TileContext): all input DMAs share one semaphore so the
instruction-level sync-wait counts stay within walrus's limits.
"""

import numpy as np

N = 33554432  # 2**25
N_CORES = 8
PER_CORE = N // N_CORES  # 4194304 elements, 16 MiB fp32
P = 128  # SBUF partitions
F = 1024  # free-dim elements per tile -> [128, 1024] fp32 = 512 KiB
N_TILES = PER_CORE // (P * F)  # 32

_CACHED = {}


def _build_nc_iters(n_iters: int, f: int = F):
    """Build the per-core program with the pipeline body repeated n_iters
    times (n_iters>1 is used only for wall-clock slope benchmarking)."""
    key = (n_iters, f)
    if key in _CACHED:
        return _CACHED[key]

    import concourse.bass as bass
    import concourse.mybir as mybir

    n_tiles = PER_CORE // (P * f)
    nc = bass.Bass()

    x = nc.declare_dram_parameter("x", [PER_CORE], mybir.dt.float32, isOutput=False)
    out = nc.declare_dram_parameter(
        "partials", [P, n_tiles], mybir.dt.float32, isOutput=True
    )
    xv = x.rearrange("(n p f) -> n p f", p=P, f=f)

    from contextlib import ExitStack

    with ExitStack() as ctx:
        buf = ctx.enter_context(
            nc.sbuf_tensor([P, n_tiles * f], mybir.dt.float32)
        )
        accs = ctx.enter_context(nc.sbuf_tensor([P, n_tiles], mybir.dt.float32))
        # One completion semaphore per tile: sem_i >= 16 iff DMA i fully
        # landed. A single shared semaphore is NOT safe here — SDMA engines
        # skew, so a count of 16*(i+1) can be reached before DMA i+1's last
        # byte lands (observed as undercounted sums at 32 tiles in flight).
        in_sems = [
            ctx.enter_context(nc.semaphore(name=f"in_sem_{i}"))
            for i in range(n_tiles)
        ]
        act_sem = ctx.enter_context(nc.semaphore())
        out_sem = ctx.enter_context(nc.semaphore())
        block = ctx.enter_context(nc.Block())

        @block.sync
        def _(sync):
            for j in range(n_iters):
                for i in range(n_tiles):
                    if j > 0:
                        # WAR: iter j's DMA into tile i must wait for
                        # iter j-1's ACT on tile i to have consumed it.
                        sync.wait_ge(act_sem, (j - 1) * n_tiles + i + 1)
                    sync.dma_start(
                        out=buf[:, i * f : (i + 1) * f], in_=xv[i]
                    ).then_inc(in_sems[i], 16)
                sync.wait_ge(act_sem, (j + 1) * n_tiles)
                sync.dma_start(out=out[:], in_=accs[:]).then_inc(out_sem, 16)
            sync.wait_ge(out_sem, 16 * n_iters)

        @block.scalar
        def _(scalar):
            for j in range(n_iters):
                if j > 0:
                    # WAR: accs is re-written each iter; previous gather
                    # DMA must have read it.
                    scalar.wait_ge(out_sem, 16 * j)
                for i in range(n_tiles):
                    scalar.wait_ge(in_sems[i], 16 * (j + 1))
                    # in-place relu; per-partition tile sum -> accs[:, i]
                    nc.scalar.activation(
                        buf[:, i * f : (i + 1) * f],
                        buf[:, i * f : (i + 1) * f],
                        mybir.ActivationFunctionType.Relu,
                        accum_out=accs[:, i : i + 1],
                    ).then_inc(act_sem, 1)

    _CACHED[key] = nc
    return nc


def _build_nc():
    return _build_nc_iters(1)


def kernel(x: np.ndarray) -> np.ndarray:
    from concourse.bass_utils import run_bass_kernel_spmd

    nc = _build_nc()

    x = np.ascontiguousarray(np.asarray(x, dtype=np.float32).reshape(-1))
    shards = x.reshape(N_CORES, PER_CORE)
    in_maps = [{"x": shards[i]} for i in range(N_CORES)]
    res = run_bass_kernel_spmd(nc, in_maps, list(range(N_CORES)))

    partials = np.stack([r["partials"] for r in res.results])  # [8, P, n_tiles]
    total = partials.astype(np.float64).sum()
    return np.asarray(max(total, 0.0), dtype=np.float32)



# revision 5
# speedup vs baseline: 3.0706x; 3.0706x over previous
"""Trainium2 Bass kernel for KANCell: relu(sum(relu(x))) over 2**25 fp32.

Data-parallel sharded reduction across 8 NeuronCores; within each core the
work is spread over four engine queues instead of one:

  - SP and Activation each stream ~1/4 of the shard HBM->SBUF as fp32
    (HWDGE dma_start is only available on SP/Act).
  - GpSimd (Pool/SWDGE) streams the remaining ~1/2 as *casting* DMAs
    (fp32 HBM -> bf16 SBUF), halving the SBUF-side bytes moved.
  - DVE consumes most chunks with a single fused
    tensor_scalar(max(x,0), +0, accum_out) = relu + per-partition sum,
    which runs in the DVE 2x mode for fp32 and 4x mode for bf16.
  - Act additionally relu+accum-consumes a couple of chunks it DMA'd
    itself (its activation-table warmup hides in its own DMA latency),
    trimming the DVE critical path.

Per-chunk semaphores (SDMA completion can skew across rings, so one shared
counting sem per queue is unsafe); each consumer waits the chunk's sem
before its fused relu+sum.  Partial sums land in one fp32 partials buffer;
SP DMAs it out after the last consumer finishes and the host does the
final (tiny) sum + ReLU.

bf16 note: half the data is summed from bf16-rounded values (round to
nearest even).  The rounding is unbiased and the final tolerance is 2e-2;
observed end-to-end relative error is ~1e-6.
"""

import numpy as np

N = 33554432  # 2**25
N_CORES = 8
PER_CORE = N // N_CORES  # 4194304 elements, 16 MiB fp32
P = 128  # SBUF partitions
W = PER_CORE // P  # 32768 elements per partition

# Chunk table: (queue, size_elems_per_partition, dtype, consumer).
# queue in {sp, act, pool}; dtype f32 on sp/act, bf16 on pool (cast DMA);
# consumer in {dve, act}.  Issue order per queue = listed order.
_SP = [544, 544, 952, 1064, 1256, 1216, 1144, 680, 608, 456]  # fp32 -> dve
_ACTD = [376, 848, 392, 824, 1120, 1232]  # fp32 -> dve
_ACTS = [1288, 1120]  # fp32 -> act (self-consumed, DMA'd last on act queue)
_POOL = [656, 944, 1600, 2024, 2136, 2344, 1264, 1280, 1840, 1224, 992, 800]  # bf16

CHUNKS = (
    [("sp", s, "f32", "dve") for s in _SP]
    + [("act", s, "f32", "dve") for s in _ACTD]
    + [("act", s, "f32", "act") for s in _ACTS]
    + [("pool", s, "bf16", "dve") for s in _POOL]
)

assert sum(c[1] for c in CHUNKS) == W, sum(c[1] for c in CHUNKS)

# v1 cost-model constants used only to pre-compute the DVE consumption
# order (correctness does not depend on them: per-chunk sems gate all
# consumers).
_DMA_NS_PER_B = 0.3855421686746988
_DMA_FLOOR = 500.0
_DMA_DELAY = {"sp": 1716.7, "act": 1716.7, "pool": 1883.3}

_CACHED = {}


def _schedule():
    """Assign HBM offsets / SBUF slots and predict land times."""
    chunks = []
    hbm_ofs = 0
    sbuf_ofs = {"f32": 0, "bf16": 0}
    busy_end = {"sp": 200.0, "act": 200.0, "pool": 200.0}
    for qname, sz, dt, consumer in CHUNKS:
        out_bytes = sz * (4 if dt == "f32" else 2)
        cost = max(_DMA_FLOOR, out_bytes * _DMA_NS_PER_B)
        busy_end[qname] += cost
        chunks.append(
            dict(
                queue=qname,
                size=sz,
                dtype=dt,
                consumer=consumer,
                hbm_ofs=hbm_ofs,
                sbuf_ofs=sbuf_ofs[dt],
                land=busy_end[qname] + _DMA_DELAY[qname],
            )
        )
        hbm_ofs += sz * P
        sbuf_ofs[dt] += sz
    assert hbm_ofs == PER_CORE
    dve_order = sorted(
        (i for i, c in enumerate(chunks) if c["consumer"] == "dve"),
        key=lambda i: chunks[i]["land"],
    )
    act_order = [i for i, c in enumerate(chunks) if c["consumer"] == "act"]
    return chunks, dve_order, act_order


def _build_nc():
    if "nc" in _CACHED:
        return _CACHED["nc"]

    import concourse.bass as bass
    import concourse.mybir as mybir
    from contextlib import ExitStack

    chunks, dve_order, act_order = _schedule()
    n_chunks = len(chunks)
    n_dve = len(dve_order)
    n_act = len(act_order)
    f32_total = sum(c["size"] for c in chunks if c["dtype"] == "f32")
    bf16_total = sum(c["size"] for c in chunks if c["dtype"] == "bf16")

    nc = bass.Bass()
    x = nc.declare_dram_parameter("x", [PER_CORE], mybir.dt.float32, isOutput=False)
    out = nc.declare_dram_parameter(
        "partials", [P, n_chunks], mybir.dt.float32, isOutput=True
    )

    with ExitStack() as ctx:
        fbuf = ctx.enter_context(nc.sbuf_tensor([P, f32_total], mybir.dt.float32))
        bbuf = ctx.enter_context(nc.sbuf_tensor([P, bf16_total], mybir.dt.bfloat16))
        accs = ctx.enter_context(nc.sbuf_tensor([P, n_chunks], mybir.dt.float32))
        in_sems = [
            ctx.enter_context(nc.semaphore(name=f"in_sem_{i}"))
            for i in range(n_chunks)
        ]
        dve_sem = ctx.enter_context(nc.semaphore(name="dve_sem"))
        act_sem = ctx.enter_context(nc.semaphore(name="act_sem"))
        out_sem = ctx.enter_context(nc.semaphore(name="out_sem"))
        block = ctx.enter_context(nc.Block())

        def tile(c):
            xin = x[c["hbm_ofs"] : c["hbm_ofs"] + c["size"] * P].rearrange(
                "(p f) -> p f", p=P, f=c["size"]
            )
            buf = fbuf if c["dtype"] == "f32" else bbuf
            dst = buf[:, c["sbuf_ofs"] : c["sbuf_ofs"] + c["size"]]
            return xin, dst

        def emit_dmas(eng, qname):
            for i, c in enumerate(chunks):
                if c["queue"] != qname:
                    continue
                xin, dst = tile(c)
                eng.dma_start(out=dst, in_=xin).then_inc(in_sems[i], 16)

        @block.sync
        def _(sync):
            emit_dmas(sync, "sp")

        @block.scalar
        def _(scalar):
            emit_dmas(scalar, "act")
            # the first activation also loads the Relu table (~1.5us); the
            # schedule accounts for it
            for k, i in enumerate(act_order):
                c = chunks[i]
                _, dst = tile(c)
                scalar.wait_ge(in_sems[i], 16)
                nc.scalar.activation(
                    dst,
                    dst,
                    mybir.ActivationFunctionType.Relu,
                    accum_out=accs[:, n_dve + k : n_dve + k + 1],
                ).then_inc(act_sem, 1)

        @block.gpsimd
        def _(g):
            emit_dmas(g, "pool")

        @block.vector
        def _(v):
            for k, i in enumerate(dve_order):
                c = chunks[i]
                _, dst = tile(c)
                v.wait_ge(in_sems[i], 16)
                nc.vector.tensor_scalar(
                    dst,
                    dst,
                    0.0,
                    0.0,
                    mybir.AluOpType.max,
                    mybir.AluOpType.add,
                    accum_out=accs[:, k : k + 1],
                ).then_inc(dve_sem, 1)

        @block.sync
        def _(sync):
            sync.wait_ge(dve_sem, n_dve)
            if n_act:
                sync.wait_ge(act_sem, n_act)
            sync.dma_start(out=out[:], in_=accs[:]).then_inc(out_sem, 16)
            sync.wait_ge(out_sem, 16)

    _CACHED["nc"] = nc
    return nc


def kernel(x: np.ndarray) -> np.ndarray:
    from concourse.bass_utils import run_bass_kernel_spmd

    nc = _build_nc()

    x = np.ascontiguousarray(np.asarray(x, dtype=np.float32).reshape(-1))
    shards = x.reshape(N_CORES, PER_CORE)
    in_maps = [{"x": shards[i]} for i in range(N_CORES)]
    res = run_bass_kernel_spmd(nc, in_maps, list(range(N_CORES)))

    partials = np.stack([r["partials"] for r in res.results])  # [8, P, n_chunks]
    total = partials.astype(np.float64).sum()
    return np.asarray(max(total, 0.0), dtype=np.float32)
